# revision 9
# baseline (speedup 1.0000x reference)
"""Trainium2 Bass kernel for nn_EnhancedAdaptiveGate.

Reference computation (per sample b of 64, channels C=128, length L=4096):
  stats = concat([mean, std, skew, diff_std, recent_mean, recent_std])  # [B, 768]
  alpha = sigmoid(gelu(gelu(stats @ W1 + b1) @ W2 + b2) @ W3 + b3)      # [B, 128]

Sharding: data-parallel over batch — 8 samples per NeuronCore, MLP weights
replicated, no cross-core communication.

Per-core schedule (hybrid, balances PE/ACT/DVE under the HBM roofline):
  - 6 samples run the folded t-major pipeline: partition p holds L-rows
    [32p, 32p+32), streams x | x^2 | x^3 | x_t*x_{t+1} side by side in one
    [128, 4, GS, C] tile (ACT fills x^2, DVE the rest), and two matmuls
    per g-chunk against a [128, 2] masked-ones stationary reduce them
    into a PSUM bank [2, 4C] (row 0: t<3072, row 1: recent).  mm_x
    (x block, depends only on the DMA) of sample i is emitted before
    mm_rest of sample i-2, so the PE queue always holds DMA-ready work.
  - 2 samples run channel-major: the bf16 stage tile is flipped by the
    DMA xbar (dma_start_transpose) to xc[c, g, p] holding t = 32p+g, so
    every per-channel sum is a free-dim reduction: ACT Copy/Square with
    accum_out yields S1/S2 (and the x^2 tensor), DVE scalar_tensor_tensor
    with accum_out yields S3 and the lag-product sums.  The recent window
    (t >= 3072) is the free-slice [:, :, 96:128]; the lag product needs no
    partition shifts at all.  This removes 2 samples of matmul streaming
    from the PE and keeps ACT/DVE busy where they were idle.
  - diff-std via the telescoping identity D2 = 2*S2 - x0^2 - xL^2 - 2*P.
  - Epilogue: PE transposes the 10 raw-sum blocks of the t-major samples
    (8 masked sums + x0|xL for all 8 samples) into [channel, quantity,
    sample]; the c-major accumulators land in the same [128, 10, 8] tile
    directly.  All stats math then runs channel-parallel on [128, 8] fp32
    tiles and writes statsT [128, 6, 8] — the layout the tiny MLP
    consumes (gelu via erf so one ACT table set covers erf+sigmoid).
"""

import numpy as np

import concourse.bass as bass
import concourse.bacc as bacc
import concourse.tile as tile
from concourse import mybir
from concourse.bass_utils import run_bass_kernel_spmd

F32 = mybir.dt.float32
BF16 = mybir.dt.bfloat16
ALU = mybir.AluOpType
ACT = mybir.ActivationFunctionType

B, L, C = 64, 4096, 128
NCORES = 8
BS = B // NCORES            # samples per core
G = 32                      # L-rows per partition (folded layout)
EPS = 1e-8

N = float(L)                # 4096
NR = float(L // 4)          # 1024
ND = float(L - 1)           # 4095

SUB = 2                     # sub-tiles per sample (pipeline granularity)
GS = G // SUB               # g-blocks per sub-tile

NT = 6                      # samples on the t-major (PE) pipeline
T_SAMPLES = list(range(NT))
CM_SAMPLES = list(range(NT, BS))
RECENT_P = 96               # c-major: t = 32p+g >= 3072  <=>  p >= 96


def _sample_dma(nc, big, s, x):
    """Start the two sub-tile cast-DMAs for t-major sample s."""
    xr = x[s].rearrange("(p g) c -> p g c", g=G)
    tiles = []
    for k in range(SUB):
        xq = big.tile([128, 4, GS, C], BF16, tag=f"xq{k % 2}", name="xq")
        nc.gpsimd.dma_start(out=xq[:, 0, :, :], in_=xr[:, k * GS:(k + 1) * GS, :])
        tiles.append(xq)
    return tiles


def _sample_elemwise(nc, tiles, bndp):
    """Fill streams 1..3 (x^2, x^3, lag product) of both sub-tiles."""
    for k in range(SUB):
        xq = tiles[k]
        nc.scalar.activation(
            out=xq[:, 1].rearrange("p g c -> p (g c)"),
            in_=xq[:, 0].rearrange("p g c -> p (g c)"),
            func=ACT.Square,
        )
        nc.vector.tensor_mul(
            out=xq[:, 2].rearrange("p g c -> p (g c)"),
            in0=xq[:, 1].rearrange("p g c -> p (g c)"),
            in1=xq[:, 0].rearrange("p g c -> p (g c)"),
        )
        nc.vector.tensor_mul(
            out=xq[:, 3, 0:GS - 1, :].rearrange("p g c -> p (g c)"),
            in0=xq[:, 0, 0:GS - 1, :].rearrange("p g c -> p (g c)"),
            in1=xq[:, 0, 1:GS, :].rearrange("p g c -> p (g c)"),
        )
    xq0, xq1 = tiles
    # cross-sub-tile lag pairs fill sub-0's last prod slot
    nc.vector.tensor_mul(xq0[:, 3, GS - 1, :], xq0[:, 0, GS - 1, :], xq1[:, 0, 0, :])
    # partition-boundary pairs x[32p+31]*x[32(p+1)] fill sub-1's last slot
    # (pair t=32p+31 has the same recent-mask as partition p); the
    # nonexistent t=L-1 pair on partition 127 is zeroed.
    bnd = bndp.tile([127, C], BF16, tag="bnd", name="bnd")
    nc.sync.dma_start(out=bnd[:], in_=xq0[1:128, 0, 0, :])
    nc.gpsimd.memset(xq1[96:128, 3, GS - 1, :], 0.0)
    nc.vector.tensor_mul(xq1[0:127, 3, GS - 1, :],
                         xq1[0:127, 0, GS - 1, :], bnd[:])


def _mm_x(nc, psA, tiles, ones2):
    n = SUB * GS
    i = 0
    for k in range(SUB):
        for g in range(GS):
            nc.tensor.matmul(psA[:, 0:C], ones2, tiles[k][:, 0, g, :],
                             start=(i == 0), stop=(i == n - 1),
                             skip_group_check=True)
            i += 1


def _mm_rest(nc, psA, tiles, ones2):
    n = SUB * GS
    i = 0
    for k in range(SUB):
        for g in range(GS):
            nc.tensor.matmul(psA.rearrange("p (s c) -> p s c", s=4)[:, 1:4, :],
                             ones2, tiles[k][:, 1:4, g, :],
                             start=(i == 0), stop=(i == n - 1),
                             skip_group_check=True)
            i += 1


def _cm_dma(nc, stgp, cmxp, s, x):
    """Load c-major sample s: cast-DMA to a t-major staging tile, then flip
    each sub-tile with the DMA xbar into xc[c, g, p] (t = 32p + g)."""
    xr = x[s].rearrange("(p g) c -> p g c", g=G)
    xc = cmxp.tile([128, G, 128], BF16, tag="xc", name="xc")
    for k in range(SUB):
        stg = stgp.tile([128, GS, C], BF16, tag=f"stg{k}", name="stg")
        nc.gpsimd.dma_start(out=stg[:], in_=xr[:, k * GS:(k + 1) * GS, :])
        nc.sync.dma_start_transpose(out=xc[:, k * GS:(k + 1) * GS, :], in_=stg[:])
    return xc


def _cm_body(nc, cmxp, col, xc, TQ):
    """All six sums of a c-major sample as free-dim accumulations.
    TQ quantity rows: S1a|S2a|S3a|Pmain|S1r|S2r|S3r|Pbnd (the epilogue only
    uses P = q3+q7, so the boundary pairs may land in q7 unsplit)."""
    A = RECENT_P
    x2c = cmxp.tile([128, G, 128], BF16, tag="x2c", name="x2c")
    dmp = cmxp.tile([128, G, 128], BF16, tag="dmp", name="dmp")

    def acc(q):
        return TQ[:, q, col:col + 1]

    # S1 via ACT Copy+accum; x2c doubles as the dump (Square rewrites it)
    nc.scalar.activation(out=x2c[:, :, 0:A], in_=xc[:, :, 0:A],
                         func=ACT.Copy, accum_out=acc(0))
    nc.scalar.activation(out=x2c[:, :, A:128], in_=xc[:, :, A:128],
                         func=ACT.Copy, accum_out=acc(4))
    # S2 via ACT Square+accum, producing the x^2 tensor
    nc.scalar.activation(out=x2c[:, :, 0:A], in_=xc[:, :, 0:A],
                         func=ACT.Square, accum_out=acc(1))
    nc.scalar.activation(out=x2c[:, :, A:128], in_=xc[:, :, A:128],
                         func=ACT.Square, accum_out=acc(5))
    # S3 = sum(x^2 * x) via DVE stt+accum
    nc.vector.scalar_tensor_tensor(out=dmp[:, :, 0:A], in0=x2c[:, :, 0:A],
                                   scalar=1.0, in1=xc[:, :, 0:A],
                                   op0=ALU.mult, op1=ALU.mult, accum_out=acc(2))
    nc.vector.scalar_tensor_tensor(out=dmp[:, :, A:128], in0=x2c[:, :, A:128],
                                   scalar=1.0, in1=xc[:, :, A:128],
                                   op0=ALU.mult, op1=ALU.mult, accum_out=acc(6))
    # P: in-partition pairs (t+1 = same p, g+1) ...
    nc.vector.scalar_tensor_tensor(out=dmp[:, 0:G - 1, :], in0=xc[:, 0:G - 1, :],
                                   scalar=1.0, in1=xc[:, 1:G, :],
                                   op0=ALU.mult, op1=ALU.mult, accum_out=acc(3))
    # ... and boundary pairs t=32p+31 -> x[32(p+1)] = (g=0, p+1)
    nc.vector.scalar_tensor_tensor(out=dmp[:, G - 1, 0:127],
                                   in0=xc[:, G - 1, 0:127], scalar=1.0,
                                   in1=xc[:, 0, 1:128],
                                   op0=ALU.mult, op1=ALU.mult, accum_out=acc(7))


def build():
    nc = bacc.Bacc("TRN2", target_bir_lowering=False, debug=False)
    x = nc.declare_dram_parameter("x", [BS, L, C], F32, isOutput=False)
    W1 = nc.declare_dram_parameter("W1", [6 * C, 128], F32, isOutput=False)
    b1 = nc.declare_dram_parameter("b1", [128], F32, isOutput=False)
    W2 = nc.declare_dram_parameter("W2", [128, 32], F32, isOutput=False)
    b2 = nc.declare_dram_parameter("b2", [32], F32, isOutput=False)
    W3 = nc.declare_dram_parameter("W3", [32, C], F32, isOutput=False)
    b3 = nc.declare_dram_parameter("b3", [C], F32, isOutput=False)
    out = nc.declare_dram_parameter("out", [C, BS], F32, isOutput=True)

    eye8 = nc.inline_tensor(np.eye(8, dtype=np.float32), name="eye8")

    with tile.TileContext(nc) as tc:
        with (
            tc.tile_pool(name="big", bufs=3) as big,
            tc.tile_pool(name="bndp", bufs=4) as bndp,
            tc.tile_pool(name="stgp", bufs=2) as stgp,
            tc.tile_pool(name="cmxp", bufs=1) as cmxp,
            tc.tile_pool(name="psum", bufs=6, space="PSUM") as psp,
            tc.tile_pool(name="stage", bufs=6) as stage,
            tc.tile_pool(name="fin", bufs=1) as fin,
            tc.tile_pool(name="pse", bufs=1, space="PSUM") as pse,
        ):
            # start sample 0 loads before any constant setup
            tiles_of = {0: _sample_dma(nc, big, T_SAMPLES[0], x)}

            ones2 = fin.tile([128, 2], BF16, tag="ones2")
            nc.vector.memset(ones2[:], 0.0)
            nc.vector.memset(ones2[0:96, 0:1], 1.0)
            nc.vector.memset(ones2[96:128, 1:2], 1.0)
            # trigger the ACT table load before the first data arrives
            warm = fin.tile([1, 8], F32, tag="warm")
            nc.vector.memset(warm[:], 0.5)
            nc.scalar.activation(out=warm[:], in_=warm[:], func=ACT.Square)
            ones1 = fin.tile([1, 8], F32, tag="ones1")
            nc.vector.memset(ones1, 1.0)
            idsb = fin.tile([8, 8], F32, tag="idsb")
            nc.sync.dma_start(out=idsb[:], in_=eye8[:])

            # MLP weights on SBUF.  W1 needs feature chunks of 128 on the
            # partitions, so load it as 6 plain [128, 128] DMAs (a single
            # strided-gather DMA is ~15us on one queue and blocks it).
            w1sb = fin.tile([128, 6, 128], F32, tag="w1sb")
            w1v = W1.rearrange("(k p) j -> k p j", p=128)
            for k in range(6):
                nc.sync.dma_start(out=w1sb[:, k, :], in_=w1v[k])
            w2sb = fin.tile([128, 32], F32, tag="w2sb")
            nc.sync.dma_start(out=w2sb[:], in_=W2[:])
            w3sb = fin.tile([32, C], F32, tag="w3sb")
            nc.sync.dma_start(out=w3sb[:], in_=W3[:])
            b1sb = fin.tile([1, 128], F32, tag="b1sb")
            nc.sync.dma_start(out=b1sb[:], in_=b1.rearrange("(a c) -> a c", a=1))
            b2sb = fin.tile([1, 32], F32, tag="b2sb")
            nc.sync.dma_start(out=b2sb[:], in_=b2.rearrange("(a c) -> a c", a=1))
            b3sb = fin.tile([1, C], F32, tag="b3sb")
            nc.sync.dma_start(out=b3sb[:], in_=b3.rearrange("(a c) -> a c", a=1))

            # Raw sums of t-major samples gathered to partition rows:
            #   raw2 blocks: S1a|S2a|S3a|Pa | S1r|S2r|S3r|Pr | x0|xL
            raw2 = fin.tile([8, 10 * C], F32, tag="raw2")
            nc.vector.memset(raw2[:, 0:8 * C], 0.0)
            nc.sync.dma_start(out=raw2[:, 8 * C:9 * C], in_=x[:, 0, :])
            nc.sync.dma_start(out=raw2[:, 9 * C:10 * C], in_=x[:, L - 1, :])

            # unified accumulator tile [channel, quantity, sample]; c-major
            # samples' accum_out land here directly, t-major columns are
            # copied in from the PE transpose of raw2.
            TQ = fin.tile([128, 10, 8], F32, tag="TQ")

            # ---------------- main loop ----------------
            psA_of = {}

            def _export(i):
                s = T_SAMPLES[i]
                stA = stage.tile([2, 4 * C], F32, tag="stA", name="stA")
                nc.scalar.copy(stA[:], psA_of.pop(i)[:])
                for r in range(2):
                    nc.sync.dma_start(
                        out=raw2[s:s + 1, 4 * C * r:4 * C * (r + 1)],
                        in_=stA[r:r + 1, :],
                    )

            cm_xc = {}
            for i in range(NT):
                if i + 1 < NT:
                    tiles_of[i + 1] = _sample_dma(nc, big, T_SAMPLES[i + 1], x)
                _sample_elemwise(nc, tiles_of[i], bndp)
                psA_of[i] = psp.tile([2, 4 * C], F32, tag="psA", name="psA")
                _mm_x(nc, psA_of[i], tiles_of[i], ones2)
                if i >= 2:
                    _mm_rest(nc, psA_of[i - 2], tiles_of[i - 2], ones2)
                if i >= 4:
                    _export(i - 4)
                # interleave the c-major samples' loads and compute
                if i == 1:
                    cm_xc[0] = _cm_dma(nc, stgp, cmxp, CM_SAMPLES[0], x)
                elif i == 2:
                    _cm_body(nc, cmxp, NT + 0, cm_xc[0], TQ)
                elif i == 3:
                    cm_xc[1] = _cm_dma(nc, stgp, cmxp, CM_SAMPLES[1], x)
                elif i == 4:
                    _cm_body(nc, cmxp, NT + 1, cm_xc[1], TQ)
            _mm_rest(nc, psA_of[NT - 2], tiles_of[NT - 2], ones2)
            _mm_rest(nc, psA_of[NT - 1], tiles_of[NT - 1], ones2)
            for i in range(NT - 4, NT):
                _export(i)

            # ------------- transpose raw sums to [channel, 10, sample] -------
            psE = pse.tile([128, 10, 8], F32, tag="pse")
            for q in range(10):
                nc.tensor.matmul(psE[:, q, :], raw2[:, q * C:(q + 1) * C], idsb[:],
                                 is_transpose=True, start=(q == 0), stop=(q == 9),
                                 skip_group_check=True)
            nc.vector.tensor_copy(TQ[:, 0:8, 0:NT], psE[:, 0:8, 0:NT])
            nc.vector.tensor_copy(TQ[:, 8:10, :], psE[:, 8:10, :])
            (t_s1a, t_s2a, t_s3a, t_pa, t_s1r, t_s2r, t_s3r, t_pr, t_x0,
             t_xl) = (TQ[:, q, :] for q in range(10))

            # ---------------- stats epilogue (channel-parallel) --------------
            # statsT rows are the MLP feature chunks: k*128 + c for stat k in
            # (mean, std, skew, diff_std, recent_mean, recent_std).
            statsT = fin.tile([128, 6, 8], F32, tag="statsT")
            SM = fin.tile([128, 4, 8], F32, tag="SM")     # S1|S2|S3|P full
            V3 = fin.tile([128, 3, 8], F32, tag="V3")     # var|rvar|dvar
            SD3 = fin.tile([128, 3, 8], F32, tag="SD3")   # sqrt of V3
            TT = fin.tile([128, 6, 8], F32, tag="TT")     # scratch
            SQ01 = fin.tile([128, 2, 8], F32, tag="SQ01")  # x0^2|xL^2

            nc.vector.tensor_add(SM[:], TQ[:, 0:4, :], TQ[:, 4:8, :])
            s1, s2, s3, pp = (SM[:, q, :] for q in range(4))
            mean, mean2, rmean, rmean2, dmean, num = (TT[:, q, :] for q in range(6))

            nc.vector.tensor_scalar_mul(out=mean, in0=s1, scalar1=1.0 / N)
            nc.vector.tensor_mul(mean2, mean, mean)
            nc.vector.scalar_tensor_tensor(out=V3[:, 0, :], in0=mean2, scalar=-N,
                                           in1=s2, op0=ALU.mult, op1=ALU.add)
            nc.vector.tensor_scalar_mul(out=rmean, in0=t_s1r, scalar1=1.0 / NR)
            nc.vector.tensor_mul(rmean2, rmean, rmean)
            nc.vector.scalar_tensor_tensor(out=V3[:, 1, :], in0=rmean2, scalar=-NR,
                                           in1=t_s2r, op0=ALU.mult, op1=ALU.add)
            nc.scalar.activation(out=SQ01.rearrange("p a b -> p (a b)"),
                                 in_=TQ[:, 8:10, :].rearrange("p a b -> p (a b)"),
                                 func=ACT.Square)
            # D2 = 2*(S2 - P) - (x0^2 + xL^2)
            nc.vector.tensor_sub(V3[:, 2, :], s2, pp)
            nc.vector.tensor_add(SQ01[:, 0, :], SQ01[:, 0, :], SQ01[:, 1, :])
            nc.vector.scalar_tensor_tensor(out=V3[:, 2, :], in0=V3[:, 2, :],
                                           scalar=2.0, in1=SQ01[:, 0, :],
                                           op0=ALU.mult, op1=ALU.subtract)
            nc.vector.tensor_sub(dmean, t_xl, t_x0)
            nc.vector.tensor_scalar_mul(out=dmean, in0=dmean, scalar1=1.0 / ND)
            nc.vector.tensor_mul(SQ01[:, 1, :], dmean, dmean)
            nc.vector.scalar_tensor_tensor(out=V3[:, 2, :], in0=SQ01[:, 1, :],
                                           scalar=-ND, in1=V3[:, 2, :],
                                           op0=ALU.mult, op1=ALU.add)
            # scale the three variance numerators by 1/(n-1)
            nc.vector.tensor_scalar_mul(out=V3[:, 0, :], in0=V3[:, 0, :],
                                        scalar1=1.0 / (N - 1))
            nc.vector.tensor_scalar_mul(out=V3[:, 1, :], in0=V3[:, 1, :],
                                        scalar1=1.0 / (NR - 1))
            nc.vector.tensor_scalar_mul(out=V3[:, 2, :], in0=V3[:, 2, :],
                                        scalar1=1.0 / (ND - 1))
            nc.scalar.activation(out=SD3.rearrange("p a b -> p (a b)"),
                                 in_=V3.rearrange("p a b -> p (a b)"),
                                 func=ACT.Sqrt)

            nc.vector.tensor_copy(statsT[:, 0, :], mean)
            nc.vector.tensor_scalar_add(out=statsT[:, 1, :], in0=SD3[:, 0, :],
                                        scalar1=EPS)
            nc.vector.tensor_copy(statsT[:, 3, :], SD3[:, 2, :])
            nc.vector.tensor_copy(statsT[:, 4, :], rmean)
            nc.vector.tensor_scalar_add(out=statsT[:, 5, :], in0=SD3[:, 1, :],
                                        scalar1=EPS)

            # skew = (S3 - 3*mean*S2 + 2*N*mean^3) / (N * std^3)
            nc.vector.tensor_mul(num, mean2, mean)                # mean^3
            nc.vector.scalar_tensor_tensor(out=num, in0=num, scalar=2.0 * N,
                                           in1=s3, op0=ALU.mult, op1=ALU.add)
            nc.vector.tensor_mul(mean2, mean, s2)                 # mean*S2
            nc.vector.scalar_tensor_tensor(out=num, in0=mean2, scalar=-3.0,
                                           in1=num, op0=ALU.mult, op1=ALU.add)
            nc.vector.reciprocal(SQ01[:, 0, :], statsT[:, 1, :])  # 1/std
            nc.vector.tensor_mul(SQ01[:, 1, :], SQ01[:, 0, :], SQ01[:, 0, :])
            nc.vector.tensor_mul(SQ01[:, 1, :], SQ01[:, 1, :], SQ01[:, 0, :])
            nc.vector.tensor_mul(num, num, SQ01[:, 1, :])
            nc.vector.tensor_scalar_mul(out=statsT[:, 2, :], in0=num,
                                        scalar1=1.0 / N)

            # ---------------- MLP (transposed: [feat, sample]) ----------------
            psH1 = pse.tile([128, 8], F32, tag="pse")
            for k in range(6):
                nc.tensor.matmul(psH1[:], w1sb[:, k, :], statsT[:, k, :],
                                 start=(k == 0), stop=False)
            nc.tensor.matmul(psH1[:], b1sb[:], ones1[:], start=False, stop=True)

            esb = fin.tile([128, 8], F32, tag="esb")
            nc.scalar.activation(out=esb[:], in_=psH1[:], func=ACT.Erf,
                                 scale=float(1.0 / np.sqrt(2.0)))
            nc.vector.tensor_scalar(out=esb[:], in0=esb[:], scalar1=1.0, scalar2=0.5,
                                    op0=ALU.add, op1=ALU.mult)
            h1sb = fin.tile([128, 8], F32, tag="h1sb")
            nc.vector.tensor_mul(h1sb[:], esb[:], psH1[:])

            psH2 = pse.tile([32, 8], F32, tag="psH2")
            nc.tensor.matmul(psH2[:], w2sb[:], h1sb[:], start=True, stop=False)
            nc.tensor.matmul(psH2[:], b2sb[:], ones1[:], start=False, stop=True)
            esb2 = fin.tile([32, 8], F32, tag="esb2")
            nc.scalar.activation(out=esb2[:], in_=psH2[:], func=ACT.Erf,
                                 scale=float(1.0 / np.sqrt(2.0)))
            nc.vector.tensor_scalar(out=esb2[:], in0=esb2[:], scalar1=1.0, scalar2=0.5,
                                    op0=ALU.add, op1=ALU.mult)
            h2sb = fin.tile([32, 8], F32, tag="h2sb")
            nc.vector.tensor_mul(h2sb[:], esb2[:], psH2[:])

            psH3 = pse.tile([128, 8], F32, tag="pse")
            nc.tensor.matmul(psH3[:], w3sb[:], h2sb[:], start=True, stop=False)
            nc.tensor.matmul(psH3[:], b3sb[:], ones1[:], start=False, stop=True)
            alphas = fin.tile([128, 8], F32, tag="alphas")
            nc.scalar.activation(out=alphas[:], in_=psH3[:], func=ACT.Sigmoid)

            nc.sync.dma_start(out=out[:], in_=alphas[:])
    nc.compile()
    return nc


_NC_CACHE = None


def _get_nc():
    global _NC_CACHE
    if _NC_CACHE is None:
        _NC_CACHE = build()
    return _NC_CACHE


def _run(inputs, **kwargs):
    x = np.ascontiguousarray(np.asarray(inputs["x"], dtype=np.float32))
    args = {k: np.ascontiguousarray(np.asarray(inputs[k], dtype=np.float32))
            for k in ("W1", "b1", "W2", "b2", "W3", "b3")}
    nc = _get_nc()
    in_maps = [dict(args, x=x[i * BS:(i + 1) * BS]) for i in range(NCORES)]
    res = run_bass_kernel_spmd(nc, in_maps, core_ids=list(range(NCORES)), **kwargs)
    out = np.concatenate([r["out"].T for r in res.results], axis=0)
    return out, res


def kernel(x, W1, b1, W2, b2, W3, b3):
    out, _ = _run(dict(x=x, W1=W1, b1=b1, W2=W2, b2=b2, W3=W3, b3=b3))
    return out
